# revision 10
# baseline (speedup 1.0000x reference)
"""Trainium2 Bass kernel for AffinityMatrixConstructLayer.

Factorization: with G/H incidence matrices ([n,e], one-hot columns),
  M[(i2,i1),(k2,k1)] = sum_{j2,j1} G2[i2,j2]H2[k2,j2] Me[j2,j1] G1[i1,j1]H1[k1,j1]
                        + diag(Mp[i2,i1])
Per i2 block-row:
  S_T   = H2T * G2T[:,i2]              [192(j2), 48(k2)]
  C2T   = Me.T @ S_T                   [192(j1), 48(k2)]
  R     = C2T[:,k2,None] * H1T[:,None,k1]   [192(j1), 2304]
  rows  = G1T.T @ R                    [48(i1), 2304]
Sharding: i2 (48 values) split 6-per-core across 8 cores. Each core's
output columns are rotated by -6c blocks (via host-shifted edge tails) so
the in-kernel diagonal add lands at a core-invariant position; the host
un-rotates and concatenates.
"""

import sys

for _p in ("/opt/trn_rl_repo", "/root/.axon_site/_ro/trn_rl_repo"):
    if _p not in sys.path:
        sys.path.insert(0, _p)

import numpy as np

import concourse.bass as bass
import concourse.mybir as mybir
from concourse.tile import TileContext
from concourse.masks import make_identity
from concourse.bass_utils import run_bass_kernel_spmd

F32 = mybir.dt.float32
F32R = mybir.dt.float32r
I32 = mybir.dt.int32
AF = mybir.ActivationFunctionType
ALU = mybir.AluOpType

N_CORES = 8
N = 48          # nodes per graph
E = 192         # edges per graph
D = 1024        # feature dim
I2P = N // N_CORES          # 6 block-rows per core
ROWS = I2P * N              # 288 output rows per core
COLS = N * N                # 2304

_CACHE: dict = {}
LAST_RESULTS = None




def _split_multiwaits(nc):
    """This walrus build encodes at most one sync-wait per instruction.
    Move extra waits onto injected single-wait drains on the same engine
    (engine queues execute in order, so semantics are preserved)."""
    for f in nc.m.functions:
        for blk in f.blocks:
            out = []
            for inst in blk.instructions:
                si = getattr(inst, "sync_info", None)
                if si is not None and si.on_wait and len(si.on_wait) > 1:
                    waits = list(si.on_wait)
                    for w in waits[:-1]:
                        d = mybir.InstDrain(
                            name=nc.get_next_instruction_name(),
                            ins=[], outs=[], bass_is_fusable=False)
                        d.engine = inst.engine
                        d.sync_info = mybir.SyncInfo(on_wait=[w], on_update=[])
                        out.append(d)
                    si.on_wait = waits[-1:]
                out.append(inst)
            try:
                blk.instructions[:] = out
            except TypeError:
                blk.instructions = out


def _softplus_relu(nc, spool, src_ap, out_ap, neghalf):
    """out = relu(softplus(src) - 0.5), stable:
    softplus(x) = relu(x) + ln(1 + exp(-|x|)); funcs all in one ACT set."""
    p, w = src_ap.shape[0], src_ap.shape[1]
    ab = spool.tile([p, w], F32, tag="sp_ab", name="sp_ab")
    nc.scalar.activation(ab, src_ap, AF.Abs)
    ex = spool.tile([p, w], F32, tag="sp_ex", name="sp_ex")
    nc.scalar.activation(ex, ab, AF.Exp, scale=-1.0)
    ln = spool.tile([p, w], F32, tag="sp_ln", name="sp_ln")
    nc.scalar.activation(ln, ex, AF.Ln, bias=1.0)
    rl = spool.tile([p, w], F32, tag="sp_rl", name="sp_rl")
    nc.scalar.activation(rl, src_ap, AF.Relu)
    pre = spool.tile([p, w], F32, tag="sp_pre", name="sp_pre")
    nc.vector.scalar_tensor_tensor(out=pre, in0=rl, scalar=-0.5, in1=ln,
                                   op0=ALU.add, op1=ALU.add)
    nc.scalar.activation(out_ap, pre, AF.Relu)


def _build() -> bass.Bass:
    if "nc" in _CACHE:
        return _CACHE["nc"]
    nc = bass.Bass(trn_type="TRN2")

    # ---- DRAM I/O ----
    d_x1r = nc.dram_tensor("x1r", [I2P, D], F32, kind="ExternalInput")
    d_x2 = nc.dram_tensor("x2", [N, D], F32, kind="ExternalInput")
    d_ef1 = nc.dram_tensor("ef1", [E, D], F32, kind="ExternalInput")
    d_ef2 = nc.dram_tensor("ef2", [E, D], F32, kind="ExternalInput")
    d_gw = nc.dram_tensor("gw", [1, D], F32, kind="ExternalInput")
    d_Wn = nc.dram_tensor("Wn", [D, D], F32, kind="ExternalInput")
    d_bn = nc.dram_tensor("bn", [D], F32, kind="ExternalInput")
    d_We = nc.dram_tensor("We", [D, D], F32, kind="ExternalInput")
    d_be = nc.dram_tensor("be", [D], F32, kind="ExternalInput")
    d_ei1 = nc.dram_tensor("ei1", [2, E], I32, kind="ExternalInput")
    d_ei2s = nc.dram_tensor("ei2s", [2, E], I32, kind="ExternalInput")
    d_out = nc.dram_tensor("out", [ROWS, COLS], F32, kind="ExternalOutput")

    KC = D // 128  # 8 contraction chunks

    with TileContext(nc) as tc:
        with (
            tc.tile_pool(name="const", bufs=1) as cpool,
            tc.tile_pool(name="wstream", bufs=3) as wpool,
            tc.tile_pool(name="scratch", bufs=2) as spool,
            tc.tile_pool(name="rbuf", bufs=2) as rpool,
            tc.tile_pool(name="orow", bufs=2) as opool,
            tc.tile_pool(name="ptr", bufs=2, space="PSUM") as ptr,
            tc.tile_pool(name="paff", bufs=2, space="PSUM") as paff,
            tc.tile_pool(name="pc2t", bufs=1, space="PSUM") as pc2t,
            tc.tile_pool(name="pout", bufs=3, space="PSUM") as pout,
        ):
            # ---------- constants ----------
            ident = cpool.tile([128, 128], F32, tag="ident", name="ident")
            make_identity(nc, ident)
            id48 = cpool.tile([N, N], F32, tag="id48", name="id48")
            make_identity(nc, id48)
            iota48 = cpool.tile([128, N], F32, tag="iota48", name="iota48")
            nc.gpsimd.iota(iota48, pattern=[[1, N]], base=0,
                           channel_multiplier=0,
                           allow_small_or_imprecise_dtypes=True)
            iota6 = cpool.tile([128, I2P], F32, tag="iota6", name="iota6")
            neghalf = cpool.tile([128, 1], F32, tag="neghalf", name="neghalf")
            nc.vector.memset(neghalf, -0.5)
            nc.gpsimd.iota(iota6, pattern=[[1, I2P]], base=0,
                           channel_multiplier=0,
                           allow_small_or_imprecise_dtypes=True)

            # edge vectors -> f32 per-partition columns (cast on SWDGE DMA)
            def edge_col(dram_row, lo, hi, tag):
                t = cpool.tile([hi - lo, 1], F32, tag=tag)
                nc.gpsimd.dma_start(out=t, in_=dram_row[lo:hi].unsqueeze(1))
                return t

            e1h = [edge_col(d_ei1[0], 0, 128, "e1h_hi"),
                   edge_col(d_ei1[0], 128, 192, "e1h_lo")]
            e1t = [edge_col(d_ei1[1], 0, 128, "e1t_hi"),
                   edge_col(d_ei1[1], 128, 192, "e1t_lo")]
            e2h = [edge_col(d_ei2s[0], 0, 128, "e2h_hi"),
                   edge_col(d_ei2s[0], 128, 192, "e2h_lo")]
            e2t = [edge_col(d_ei2s[1], 0, 128, "e2t_hi"),
                   edge_col(d_ei2s[1], 128, 192, "e2t_lo")]

            # incidence (transposed): X[j, node] = (edge_val[j] == node)
            def incid(evec, width, iota, tag, dt=F32):
                tiles = []
                for ci, ev in enumerate(evec):
                    p = ev.shape[0]
                    t = cpool.tile([p, width], dt, tag=f"{tag}{ci}", name=f"{tag}{ci}")
                    nc.vector.tensor_tensor(t, iota[0:p, :],
                                            ev.broadcast_to((p, width)),
                                            ALU.is_equal)
                    tiles.append(t)
                return tiles

            G1T = incid(e1h, N, iota48, "G1T", dt=F32R)
            H1T = incid(e1t, N, iota48, "H1T")
            G2T6 = incid(e2h, I2P, iota6, "G2T6")
            H2T = incid(e2t, N, iota48, "H2T")

            # bias / gw
            bn_t = cpool.tile([128, KC], F32, tag="bn", name="bn")
            nc.sync.dma_start(out=bn_t, in_=d_bn[:].rearrange("(c p) -> p c", p=128))
            be_t = cpool.tile([128, KC], F32, tag="be", name="be")
            nc.sync.dma_start(out=be_t, in_=d_be[:].rearrange("(c p) -> p c", p=128))
            gw_row = cpool.tile([1, D], F32, tag="gw_row", name="gw_row")
            nc.sync.dma_start(out=gw_row, in_=d_gw[:, :])
            gw_b = cpool.tile([128, D], F32, tag="gw_b", name="gw_b")
            nc.sync.dma_start(out=gw_b, in_=d_gw[:, :].broadcast_to((128, D)))

            # ---------- matvec  coeff = tanh(W @ gw + b) ----------
            mv = {}
            for nm, dmat in (("n", d_Wn), ("e", d_We)):
                mvt = cpool.tile([128, KC], F32, tag=f"mv_{nm}", name=f"mv_{nm}")
                mv[nm] = mvt
                for k in range(KC):
                    wt = wpool.tile([128, D], F32, tag="w", name="w")
                    nc.sync.dma_start(out=wt, in_=dmat[k * 128:(k + 1) * 128, :])
                    sc = spool.tile([128, D], F32, tag="ttr_out", name="ttr_out")
                    nc.vector.scalar_tensor_tensor(
                        out=sc, in0=wt, scalar=0.0, in1=gw_b,
                        op0=ALU.bypass, op1=ALU.mult,
                        accum_out=mvt[:, k:k + 1])
            # tanh(v) = 1 - 2/(exp(2v)+1)  (avoids the tanh ACT table)
            coeff_n = cpool.tile([128, KC], F32, tag="coeff_n", name="coeff_n")
            coeff_e = cpool.tile([128, KC], F32, tag="coeff_e", name="coeff_e")
            for nm, bias_t, cf in (("n", bn_t, coeff_n), ("e", be_t, coeff_e)):
                mvb = spool.tile([128, KC], F32, tag="mvb", name="mvb")
                nc.vector.tensor_add(mvb, mv[nm], bias_t)
                et = spool.tile([128, KC], F32, tag="et", name="et")
                nc.scalar.activation(et, mvb, AF.Exp, scale=2.0)
                nc.vector.tensor_scalar_add(et, et, 1.0)
                rt = spool.tile([128, KC], F32, tag="rt", name="rt")
                nc.vector.reciprocal(rt, et)
                nc.vector.tensor_scalar(cf, rt, -2.0, 1.0, ALU.mult, ALU.add)

            # ---------- natural-layout loads ----------
            x2_sb = cpool.tile([N, D], F32, tag="x2_sb", name="x2_sb")
            nc.sync.dma_start(out=x2_sb, in_=d_x2[:, :])
            x1r_sb = cpool.tile([I2P, D], F32, tag="x1r_sb", name="x1r_sb")
            nc.sync.dma_start(out=x1r_sb, in_=d_x1r[:, :])
            ef1_sb = [cpool.tile([128, D], F32, tag="ef1_hi", name="ef1_hi"),
                      cpool.tile([64, D], F32, tag="ef1_lo", name="ef1_lo")]
            nc.sync.dma_start(out=ef1_sb[0], in_=d_ef1[0:128, :])
            nc.sync.dma_start(out=ef1_sb[1], in_=d_ef1[128:192, :])
            ef2_sb = [cpool.tile([128, D], F32, tag="ef2_hi", name="ef2_hi"),
                      cpool.tile([64, D], F32, tag="ef2_lo", name="ef2_lo")]
            nc.sync.dma_start(out=ef2_sb[0], in_=d_ef2[0:128, :])
            nc.sync.dma_start(out=ef2_sb[1], in_=d_ef2[128:192, :])

            # ---------- transposes (PE) + scaled copies ----------
            x2T = cpool.tile([128, KC * N], F32, tag="x2T", name="x2T")
            a1T = cpool.tile([128, KC * I2P], F32, tag="a1T", name="a1T")
            ef2T = cpool.tile([128, KC * E], F32, tag="ef2T", name="ef2T")
            a_ef1T = cpool.tile([128, KC * E], F32, tag="a_ef1T", name="a_ef1T")
            for k in range(KC):
                s = slice(k * 128, (k + 1) * 128)
                pt = ptr.tile([128, N], F32, tag="tr", name="tr")
                nc.tensor.transpose(pt, x2_sb[:, s], ident[0:N, 0:N])
                nc.scalar.copy(x2T[:, k * N:(k + 1) * N], pt)

                pt = ptr.tile([128, I2P], F32, tag="tr", name="tr")
                nc.tensor.transpose(pt, x1r_sb[:, s], ident[0:I2P, 0:I2P])
                nc.scalar.activation(a1T[:, k * I2P:(k + 1) * I2P], pt,
                                     AF.Copy, scale=coeff_n[:, k:k + 1])

                pt = ptr.tile([128, E], F32, tag="tr", name="tr")
                nc.tensor.transpose(pt[:, 0:128], ef2_sb[0][:, s], ident)
                nc.tensor.transpose(pt[:, 128:192], ef2_sb[1][:, s],
                                    ident[0:64, 0:64])
                nc.scalar.copy(ef2T[:, k * E:(k + 1) * E], pt)

                pt = ptr.tile([128, E], F32, tag="tr", name="tr")
                nc.tensor.transpose(pt[:, 0:128], ef1_sb[0][:, s], ident)
                nc.tensor.transpose(pt[:, 128:192], ef1_sb[1][:, s],
                                    ident[0:64, 0:64])
                nc.scalar.activation(a_ef1T[:, k * E:(k + 1) * E], pt,
                                     AF.Copy, scale=coeff_e[:, k:k + 1])

            # ---------- Mp (rows for this core) ----------
            pn = paff.tile([I2P, N], F32, tag="affe", name="affe")
            for k in range(KC):
                nc.tensor.matmul(pn, a1T[:, k * I2P:(k + 1) * I2P],
                                 x2T[:, k * N:(k + 1) * N],
                                 start=(k == 0), stop=(k == KC - 1))
            mp_mine = cpool.tile([I2P, N], F32, tag="mp_mine", name="mp_mine")
            _softplus_relu(nc, spool, pn, mp_mine, neghalf)
            ptm = ptr.tile([N, I2P], F32, tag="tr", name="tr")
            nc.tensor.transpose(ptm, mp_mine, ident[0:I2P, 0:I2P])
            mpT = cpool.tile([N, I2P], F32, tag="mpT", name="mpT")
            nc.scalar.copy(mpT, ptm)

            # ---------- Me ----------
            me = [cpool.tile([128, E], F32, tag="me_hi", name="me_hi"),
                  cpool.tile([64, E], F32, tag="me_lo", name="me_lo")]
            for mi, (mlo, mhi) in enumerate(((0, 128), (128, 192))):
                pe_ = paff.tile([mhi - mlo, E], F32, tag="affe", name="affe")
                for k in range(KC):
                    nc.tensor.matmul(
                        pe_, a_ef1T[:, k * E + mlo:k * E + mhi],
                        ef2T[:, k * E:(k + 1) * E],
                        start=(k == 0), stop=(k == KC - 1))
                _softplus_relu(nc, spool, pe_, me[mi], neghalf)

            # ---------- per-i2 block rows ----------
            NT = [(t * 512, min(COLS, (t + 1) * 512)) for t in range((COLS + 511) // 512)]
            for i2 in range(I2P):
                S = []
                for ci, p in ((0, 128), (1, 64)):
                    st = spool.tile([p, N], F32, tag=f"S{ci}", name=f"S{ci}")
                    nc.vector.tensor_scalar_mul(st, H2T[ci],
                                                G2T6[ci][:, i2:i2 + 1])
                    S.append(st)
                cps = pc2t.tile([128, 2 * N], F32, tag="c2t", name="c2t")
                # C2T rows 0:128  (j1 hi)
                nc.tensor.matmul(cps[:, 0:N], me[0][:, 0:128], S[0],
                                 start=True, stop=False)
                nc.tensor.matmul(cps[:, 0:N], me[1][:, 0:128], S[1],
                                 start=False, stop=True)
                # C2T rows 128:192 (j1 lo)
                nc.tensor.matmul(cps[0:64, N:2 * N], me[0][:, 128:192], S[0],
                                 start=True, stop=False)
                nc.tensor.matmul(cps[0:64, N:2 * N], me[1][:, 128:192], S[1],
                                 start=False, stop=True)
                c2t_hi = spool.tile([128, N], F32, tag="c2t_hi", name="c2t_hi")
                nc.scalar.copy(c2t_hi, cps[:, 0:N])
                c2t_lo = spool.tile([64, N], F32, tag="c2t_lo", name="c2t_lo")
                nc.scalar.copy(c2t_lo, cps[0:64, N:2 * N])

                r_hi = rpool.tile([128, COLS], F32R, tag="R_hi", name="R_hi")
                nc.vector.tensor_mul(
                    r_hi.rearrange("p (a b) -> p a b", b=N),
                    c2t_hi.unsqueeze(2).broadcast_to((128, N, N)),
                    H1T[0].unsqueeze(1).broadcast_to((128, N, N)))
                r_lo = rpool.tile([64, COLS], F32R, tag="R_lo", name="R_lo")
                nc.gpsimd.tensor_mul(
                    r_lo.rearrange("p (a b) -> p a b", b=N),
                    c2t_lo.unsqueeze(2).broadcast_to((64, N, N)),
                    H1T[1].unsqueeze(1).broadcast_to((64, N, N)))

                orow = opool.tile([N, COLS], F32, tag="orow", name="orow")
                for t0, t1 in NT:
                    ps = pout.tile([N, 512], F32, tag="po", name="po")
                    w = t1 - t0
                    nc.tensor.matmul(ps[:, 0:w], G1T[0],
                                     r_hi[:, t0:t1],
                                     start=True, stop=False)
                    nc.tensor.matmul(ps[:, 0:w], G1T[1],
                                     r_lo[:, t0:t1],
                                     start=False, stop=True)
                    nc.scalar.copy(orow[:, t0:t1], ps[:, 0:w])
                # diagonal add at core-invariant block i2
                dcol = slice(i2 * N, (i2 + 1) * N)
                nc.vector.scalar_tensor_tensor(
                    out=orow[:, dcol], in0=id48, scalar=mpT[:, i2:i2 + 1],
                    in1=orow[:, dcol], op0=ALU.mult, op1=ALU.add)
                nc.sync.dma_start(out=d_out[i2 * N:(i2 + 1) * N, :], in_=orow)

    _split_multiwaits(nc)
    _CACHE["nc"] = nc
    return nc


def kernel(**inputs) -> np.ndarray:
    global LAST_RESULTS
    nc = _build()
    a = {k: np.ascontiguousarray(np.asarray(v)) for k, v in inputs.items()}
    ei2 = a["edge_index2"].astype(np.int32)

    in_maps = []
    for c in range(N_CORES):
        ei2s = np.stack([
            ei2[0] - I2P * c,                    # heads, shifted (match 0..5)
            (ei2[1] - I2P * c) % N,              # tails, rotated
        ]).astype(np.int32)
        in_maps.append({
            "x1r": a["x1"][I2P * c:I2P * (c + 1)],
            "x2": a["x2"],
            "ef1": a["ef1"],
            "ef2": a["ef2"],
            "gw": a["global_weight"].reshape(1, D),
            "Wn": a["Wn"],
            "bn": a["bn"],
            "We": a["We"],
            "be": a["be"],
            "ei1": a["edge_index1"].astype(np.int32),
            "ei2s": ei2s,
        })

    res = run_bass_kernel_spmd(nc, in_maps, core_ids=list(range(N_CORES)))
    LAST_RESULTS = res

    parts = []
    for c in range(N_CORES):
        o = res.results[c]["out"].reshape(ROWS, N, N)
        parts.append(np.roll(o, I2P * c, axis=1).reshape(ROWS, COLS))
    return np.concatenate(parts, axis=0).astype(np.float32)


if __name__ == "__main__":
    _build()
    print("build OK")


# revision 15
# speedup vs baseline: 1.0487x; 1.0487x over previous
"""Trainium2 Bass kernel for AffinityMatrixConstructLayer (v2, sharded).

Factorization: with G/H incidence matrices ([n,e], one-hot columns),
  M[(i2,i1),(k2,k1)] = sum_{j2,j1} G2[i2,j2]H2[k2,j2] Me[j2,j1] G1[i1,j1]H1[k1,j1]
                        + diag(Mp[i2,i1])
Per i2 block-row:
  S_T   = H2T * G2T[:,i2]              [192(j2), 48(k2)]
  C2T   = Me.T @ S_T                   [192(j1), 48(k2)]
  R     = C2T[:,k2,None] * H1T[:,None,k1]   [192(j1), 2304]
  rows  = G1T.T @ R                    [48(i1), 2304]

Sharding:
 - The d=1024 feature contraction for the affinity matrices is sharded
   128-per-core (each core holds a 128-row slice of Wn/We and 128-column
   slices of x1/x2/ef1/ef2); partial affinities are AllReduce-summed
   (153 KB) and everything downstream is computed per-core.
 - The 48 i2 block-rows are split 6-per-core. Each core's output columns
   are rotated by -6c blocks (via host-rotated edge tails) so the
   in-kernel diagonal add lands at a core-invariant position; the host
   un-rotates and concatenates. A host-passed one-hot `sel` [48,6]
   selects the core's Mp columns via a matmul.
"""

import sys

for _p in ("/opt/trn_rl_repo", "/root/.axon_site/_ro/trn_rl_repo"):
    if _p not in sys.path:
        sys.path.insert(0, _p)

import numpy as np

import concourse.bass as bass
import concourse.mybir as mybir
from concourse.tile import TileContext
from concourse.masks import make_identity
from concourse.bass_utils import run_bass_kernel_spmd

F32 = mybir.dt.float32
F32R = mybir.dt.float32r
I32 = mybir.dt.int32
AF = mybir.ActivationFunctionType
ALU = mybir.AluOpType

N_CORES = 8
N = 48          # nodes per graph
E = 192         # edges per graph
D = 1024        # feature dim
DS = D // N_CORES           # 128 feature dims per core
I2P = N // N_CORES          # 6 block-rows per core
ROWS = I2P * N              # 288 output rows per core
COLS = N * N                # 2304
AE_OFF = N * N
CC_LEN = N * N + E * E      # 39168

_CACHE: dict = {}
LAST_RESULTS = None


def _split_multiwaits(nc):
    """This walrus build encodes at most one sync-wait per instruction.
    Move extra waits onto injected single-wait drains on the same engine
    (engine queues execute in order, so semantics are preserved)."""
    for f in nc.m.functions:
        for blk in f.blocks:
            out = []
            for inst in blk.instructions:
                si = getattr(inst, "sync_info", None)
                if si is not None and si.on_wait and len(si.on_wait) > 1:
                    waits = list(si.on_wait)
                    for w in waits[:-1]:
                        d = mybir.InstDrain(
                            name=nc.get_next_instruction_name(),
                            ins=[], outs=[], bass_is_fusable=False)
                        d.engine = inst.engine
                        d.sync_info = mybir.SyncInfo(on_wait=[w], on_update=[])
                        out.append(d)
                    si.on_wait = waits[-1:]
                out.append(inst)
            try:
                blk.instructions[:] = out
            except TypeError:
                blk.instructions = out


def _softplus_relu(nc, spool, src_ap, out_ap, neghalf):
    """out = relu(softplus(src) - 0.5), stable:
    softplus(x) = relu(x) + ln(1 + exp(-|x|)); funcs all in one ACT set."""
    p, w = src_ap.shape[0], src_ap.shape[1]
    ab = spool.tile([p, w], F32, tag="sp_ab", name="sp_ab")
    nc.scalar.activation(ab, src_ap, AF.Abs)
    ex = spool.tile([p, w], F32, tag="sp_ex", name="sp_ex")
    nc.scalar.activation(ex, ab, AF.Exp, scale=-1.0)
    ln = spool.tile([p, w], F32, tag="sp_ln", name="sp_ln")
    nc.scalar.activation(ln, ex, AF.Ln, bias=1.0)
    rl = spool.tile([p, w], F32, tag="sp_rl", name="sp_rl")
    nc.scalar.activation(rl, src_ap, AF.Relu)
    pre = spool.tile([p, w], F32, tag="sp_pre", name="sp_pre")
    nc.vector.scalar_tensor_tensor(out=pre, in0=rl, scalar=-0.5, in1=ln,
                                   op0=ALU.add, op1=ALU.add)
    nc.scalar.activation(out_ap, pre, AF.Relu)


def _build() -> bass.Bass:
    if "nc" in _CACHE:
        return _CACHE["nc"]
    nc = bass.Bass(trn_type="TRN2", num_devices=N_CORES)

    d_Wns = nc.dram_tensor("Wns", [DS, D], F32, kind="ExternalInput")
    d_Wes = nc.dram_tensor("Wes", [DS, D], F32, kind="ExternalInput")
    d_gw = nc.dram_tensor("gw", [1, D], F32, kind="ExternalInput")
    d_bns = nc.dram_tensor("bns", [DS, 1], F32, kind="ExternalInput")
    d_bes = nc.dram_tensor("bes", [DS, 1], F32, kind="ExternalInput")
    d_x1s = nc.dram_tensor("x1s", [N, DS], F32, kind="ExternalInput")
    d_x2s = nc.dram_tensor("x2s", [N, DS], F32, kind="ExternalInput")
    d_ef1s = nc.dram_tensor("ef1s", [E, DS], F32, kind="ExternalInput")
    d_ef2s = nc.dram_tensor("ef2s", [E, DS], F32, kind="ExternalInput")
    d_ei1 = nc.dram_tensor("ei1", [2, E], I32, kind="ExternalInput")
    d_ei2s = nc.dram_tensor("ei2s", [2, E], I32, kind="ExternalInput")
    d_sel = nc.dram_tensor("sel", [N, I2P], F32, kind="ExternalInput")
    d_out = nc.dram_tensor("out", [ROWS, COLS], F32, kind="ExternalOutput")
    d_ccin = nc.dram_tensor("ccin", [CC_LEN], F32)
    d_ccout = nc.dram_tensor("ccout", [CC_LEN], F32)

    with TileContext(nc) as tc:
        with (
            tc.tile_pool(name="const", bufs=1) as cpool,
            tc.tile_pool(name="scratch", bufs=2) as spool,
            tc.tile_pool(name="rbuf", bufs=2) as rpool,
            tc.tile_pool(name="orow", bufs=2) as opool,
            tc.tile_pool(name="ptr", bufs=1, space="PSUM") as ptr,
            tc.tile_pool(name="paff", bufs=1, space="PSUM") as paff,
            tc.tile_pool(name="pc2t", bufs=1, space="PSUM") as pc2t,
            tc.tile_pool(name="pout", bufs=3, space="PSUM") as pout,
        ):
            # ---------- constants ----------
            ident = cpool.tile([128, 128], F32, tag="ident", name="ident")
            make_identity(nc, ident)
            id48 = cpool.tile([N, N], F32, tag="id48", name="id48")
            make_identity(nc, id48)
            iota48 = cpool.tile([128, N], F32, tag="iota48", name="iota48")
            nc.gpsimd.iota(iota48, pattern=[[1, N]], base=0,
                           channel_multiplier=0,
                           allow_small_or_imprecise_dtypes=True)
            iota6 = cpool.tile([128, I2P], F32, tag="iota6", name="iota6")
            nc.gpsimd.iota(iota6, pattern=[[1, I2P]], base=0,
                           channel_multiplier=0,
                           allow_small_or_imprecise_dtypes=True)
            neghalf = cpool.tile([128, 1], F32, tag="neghalf", name="neghalf")
            nc.vector.memset(neghalf, -0.5)

            # edge vectors -> f32 per-partition columns (cast on SWDGE DMA)
            ev_tiles = {}
            for tag, dt_ in (("e1", d_ei1), ("e2", d_ei2s)):
                for ci, (lo, hi) in enumerate(((0, 128), (128, 192))):
                    t = cpool.tile([hi - lo, 2], F32, tag=f"{tag}_{ci}",
                                   name=f"{tag}_{ci}")
                    nc.gpsimd.dma_start(
                        out=t, in_=dt_[:, lo:hi].rearrange("a b -> b a"))
                    ev_tiles[(tag, ci)] = t

            # incidence (transposed): X[j, node] = (edge_val[j] == node)
            def incid(tag_src, col, width, iota, tag, dt=F32):
                tiles = []
                for ci, p in ((0, 128), (1, 64)):
                    ev = ev_tiles[(tag_src, ci)][:, col:col + 1]
                    t = cpool.tile([p, width], dt, tag=f"{tag}{ci}",
                                   name=f"{tag}{ci}")
                    nc.vector.tensor_tensor(t, iota[0:p, :],
                                            ev.broadcast_to((p, width)),
                                            ALU.is_equal)
                    tiles.append(t)
                return tiles

            G1T = incid("e1", 0, N, iota48, "G1T", dt=F32R)
            H1T = incid("e1", 1, N, iota48, "H1T")
            G2T6 = incid("e2", 0, I2P, iota6, "G2T6")
            H2T = incid("e2", 1, N, iota48, "H2T")

            # H1T tiled 48x along the free dim (constant across i2)
            h1tiled = []
            for ci, p in ((0, 128), (1, 64)):
                ht = cpool.tile([p, COLS], F32, tag=f"h1tl{ci}",
                                name=f"h1tl{ci}")
                nc.vector.tensor_copy(
                    ht.rearrange("p (a b) -> p a b", b=N),
                    H1T[ci].unsqueeze(1).broadcast_to((p, N, N)))
                h1tiled.append(ht)

            # ---------- W shard loads + matvec + tanh ----------
            gw_b = cpool.tile([128, D], F32, tag="gw_b", name="gw_b")
            nc.sync.dma_start(out=gw_b, in_=d_gw[:, :].broadcast_to((128, D)))
            bn_t = cpool.tile([DS, 1], F32, tag="bn", name="bn")
            nc.sync.dma_start(out=bn_t, in_=d_bns[:, :])
            be_t = cpool.tile([DS, 1], F32, tag="be", name="be")
            nc.sync.dma_start(out=be_t, in_=d_bes[:, :])

            coeff = {}
            for nm, dmat, bias_t in (("n", d_Wns, bn_t), ("e", d_Wes, be_t)):
                wt = spool.tile([DS, D], F32, tag=f"w{nm}", name=f"w{nm}")
                nc.sync.dma_start(out=wt, in_=dmat[:, :])
                sc = spool.tile([DS, D], F32, tag="sttout", name="sttout")
                mvt = cpool.tile([DS, 1], F32, tag=f"mv{nm}", name=f"mv{nm}")
                nc.vector.scalar_tensor_tensor(
                    out=sc, in0=wt, scalar=0.0, in1=gw_b,
                    op0=ALU.bypass, op1=ALU.mult, accum_out=mvt)
                # tanh(v) = 1 - 2/(exp(2v)+1)
                mvb = spool.tile([DS, 1], F32, tag="mvb", name="mvb")
                nc.vector.tensor_add(mvb, mvt, bias_t)
                et = spool.tile([DS, 1], F32, tag="et", name="et")
                nc.scalar.activation(et, mvb, AF.Exp, scale=2.0)
                nc.vector.tensor_scalar_add(et, et, 1.0)
                rt = spool.tile([DS, 1], F32, tag="rt", name="rt")
                nc.vector.reciprocal(rt, et)
                cf = cpool.tile([DS, 1], F32, tag=f"coeff{nm}",
                                name=f"coeff{nm}")
                nc.vector.tensor_scalar(cf, rt, -2.0, 1.0, ALU.mult, ALU.add)
                coeff[nm] = cf

            # ---------- feature-slice loads + transposes ----------
            x1_sb = cpool.tile([N, DS], F32, tag="x1_sb", name="x1_sb")
            nc.sync.dma_start(out=x1_sb, in_=d_x1s[:, :])
            x2_sb = cpool.tile([N, DS], F32, tag="x2_sb", name="x2_sb")
            nc.sync.dma_start(out=x2_sb, in_=d_x2s[:, :])
            ef1_sb = [cpool.tile([128, DS], F32, tag="ef1_hi", name="ef1_hi"),
                      cpool.tile([64, DS], F32, tag="ef1_lo", name="ef1_lo")]
            nc.sync.dma_start(out=ef1_sb[0], in_=d_ef1s[0:128, :])
            nc.sync.dma_start(out=ef1_sb[1], in_=d_ef1s[128:192, :])
            ef2_sb = [cpool.tile([128, DS], F32, tag="ef2_hi", name="ef2_hi"),
                      cpool.tile([64, DS], F32, tag="ef2_lo", name="ef2_lo")]
            nc.sync.dma_start(out=ef2_sb[0], in_=d_ef2s[0:128, :])
            nc.sync.dma_start(out=ef2_sb[1], in_=d_ef2s[128:192, :])
            sel_sb = cpool.tile([N, I2P], F32, tag="sel_sb", name="sel_sb")
            nc.sync.dma_start(out=sel_sb, in_=d_sel[:, :])

            pt1 = ptr.tile([128, N], F32, tag="tr", name="pt1")
            nc.tensor.transpose(pt1, x1_sb, ident[0:N, 0:N])
            a1T = cpool.tile([128, N], F32, tag="a1T", name="a1T")
            nc.scalar.activation(a1T, pt1, AF.Copy, scale=coeff["n"])

            pt2 = ptr.tile([128, N], F32, tag="tr", name="pt2")
            nc.tensor.transpose(pt2, x2_sb, ident[0:N, 0:N])
            x2T = cpool.tile([128, N], F32, tag="x2T", name="x2T")
            nc.scalar.copy(x2T, pt2)

            pt3 = ptr.tile([128, E], F32, tag="tre", name="pt3")
            nc.tensor.transpose(pt3[:, 0:128], ef1_sb[0], ident)
            nc.tensor.transpose(pt3[:, 128:192], ef1_sb[1], ident[0:64, 0:64])
            aef1T = cpool.tile([128, E], F32, tag="aef1T", name="aef1T")
            nc.scalar.activation(aef1T, pt3, AF.Copy, scale=coeff["e"])

            pt4 = ptr.tile([128, E], F32, tag="tre", name="pt4")
            nc.tensor.transpose(pt4[:, 0:128], ef2_sb[0], ident)
            nc.tensor.transpose(pt4[:, 128:192], ef2_sb[1], ident[0:64, 0:64])
            ef2T = cpool.tile([128, E], F32, tag="ef2T", name="ef2T")
            nc.scalar.copy(ef2T, pt4)

            # ---------- partial affinities + AllReduce ----------
            pn = paff.tile([N, N], F32, tag="pa", name="pn")
            nc.tensor.matmul(pn, a1T, x2T, start=True, stop=True)
            pn_sb = spool.tile([N, N], F32, tag="pn_sb", name="pn_sb")
            nc.scalar.copy(pn_sb, pn)
            nc.sync.dma_start(
                out=d_ccin[0:AE_OFF].rearrange("(p f) -> p f", p=N),
                in_=pn_sb)
            for mi, (mlo, mhi) in enumerate(((0, 128), (128, 192))):
                pe_ = paff.tile([mhi - mlo, E], F32, tag="pa", name="pe_")
                nc.tensor.matmul(pe_, aef1T[:, mlo:mhi], ef2T,
                                 start=True, stop=True)
                pe_sb = spool.tile([mhi - mlo, E], F32, tag="pe_sb",
                                   name="pe_sb")
                nc.scalar.copy(pe_sb, pe_)
                off = AE_OFF + mlo * E
                nc.sync.dma_start(
                    out=d_ccin[off:off + (mhi - mlo) * E].rearrange(
                        "(p f) -> p f", p=mhi - mlo),
                    in_=pe_sb)
            nc.gpsimd.collective_compute(
                "AllReduce", ALU.add,
                replica_groups=[list(range(N_CORES))],
                ins=[d_ccin[:].rearrange("(a b) -> a b", b=128)],
                outs=[d_ccout[:].rearrange("(a b) -> a b", b=128)])

            an_s = cpool.tile([N, N], F32, tag="an_s", name="an_s")
            nc.sync.dma_start(
                out=an_s,
                in_=d_ccout[0:AE_OFF].rearrange("(p f) -> p f", p=N))
            ae_s = [cpool.tile([128, E], F32, tag="ae_hi", name="ae_hi"),
                    cpool.tile([64, E], F32, tag="ae_lo", name="ae_lo")]
            nc.sync.dma_start(
                out=ae_s[0],
                in_=d_ccout[AE_OFF:AE_OFF + 128 * E].rearrange(
                    "(p f) -> p f", p=128))
            nc.sync.dma_start(
                out=ae_s[1],
                in_=d_ccout[AE_OFF + 128 * E:].rearrange(
                    "(p f) -> p f", p=64))

            # ---------- nonlinearities ----------
            mp = cpool.tile([N, N], F32, tag="mp", name="mp")
            _softplus_relu(nc, spool, an_s, mp, neghalf)
            me = [cpool.tile([128, E], F32, tag="me_hi", name="me_hi"),
                  cpool.tile([64, E], F32, tag="me_lo", name="me_lo")]
            _softplus_relu(nc, spool, ae_s[0], me[0], neghalf)
            _softplus_relu(nc, spool, ae_s[1], me[1], neghalf)

            # Mp columns for this core via one-hot sel: [48(i1), 6(i2l)]
            pmp = ptr.tile([N, I2P], F32, tag="tr", name="pmp")
            nc.tensor.matmul(pmp, mp, sel_sb, start=True, stop=True)
            mpT = cpool.tile([N, I2P], F32, tag="mpT", name="mpT")
            nc.scalar.copy(mpT, pmp)

            # ---------- per-i2 block rows ----------
            NT = [(t * 512, min(COLS, (t + 1) * 512))
                  for t in range((COLS + 511) // 512)]
            for i2 in range(I2P):
                S = []
                for ci, p in ((0, 128), (1, 64)):
                    st = spool.tile([p, N], F32, tag=f"S{ci}", name=f"S{ci}")
                    nc.vector.tensor_scalar_mul(st, H2T[ci],
                                                G2T6[ci][:, i2:i2 + 1])
                    S.append(st)
                cps = pc2t.tile([128, 2 * N], F32, tag="c2t", name="cps")
                nc.tensor.matmul(cps[:, 0:N], me[0][:, 0:128], S[0],
                                 start=True, stop=False)
                nc.tensor.matmul(cps[:, 0:N], me[1][:, 0:128], S[1],
                                 start=False, stop=True)
                nc.tensor.matmul(cps[0:64, N:2 * N], me[0][:, 128:192], S[0],
                                 start=True, stop=False)
                nc.tensor.matmul(cps[0:64, N:2 * N], me[1][:, 128:192], S[1],
                                 start=False, stop=True)
                c2t_hi = spool.tile([128, N], F32, tag="c2t_hi", name="c2t_hi")
                nc.scalar.copy(c2t_hi, cps[:, 0:N])
                c2t_lo = spool.tile([64, N], F32, tag="c2t_lo", name="c2t_lo")
                nc.scalar.copy(c2t_lo, cps[0:64, N:2 * N])

                r_hi = rpool.tile([128, COLS], F32R, tag="R_hi", name="r_hi")
                nc.vector.tensor_mul(
                    r_hi.rearrange("p (a b) -> p a b", b=N),
                    h1tiled[0].rearrange("p (a b) -> p a b", b=N),
                    c2t_hi.unsqueeze(2).broadcast_to((128, N, N)))
                r_lo = rpool.tile([64, COLS], F32R, tag="R_lo", name="r_lo")
                nc.vector.tensor_mul(
                    r_lo.rearrange("p (a b) -> p a b", b=N),
                    h1tiled[1].rearrange("p (a b) -> p a b", b=N),
                    c2t_lo.unsqueeze(2).broadcast_to((64, N, N)))

                orow = opool.tile([N, COLS], F32, tag="orow", name="orow")
                for t0, t1 in NT:
                    ps = pout.tile([N, 512], F32, tag="po", name="ps")
                    w = t1 - t0
                    nc.tensor.matmul(ps[:, 0:w], G1T[0], r_hi[:, t0:t1],
                                     start=True, stop=False)
                    nc.tensor.matmul(ps[:, 0:w], G1T[1], r_lo[:, t0:t1],
                                     start=False, stop=True)
                    nc.scalar.copy(orow[:, t0:t1], ps[:, 0:w])
                dcol = slice(i2 * N, (i2 + 1) * N)
                nc.vector.scalar_tensor_tensor(
                    out=orow[:, dcol], in0=id48, scalar=mpT[:, i2:i2 + 1],
                    in1=orow[:, dcol], op0=ALU.mult, op1=ALU.add)
                nc.sync.dma_start(out=d_out[i2 * N:(i2 + 1) * N, :], in_=orow)

    _split_multiwaits(nc)
    _CACHE["nc"] = nc
    return nc


def _make_in_maps(a):
    ei2 = a["edge_index2"].astype(np.int32)
    eye = np.eye(N, dtype=np.float32)
    in_maps = []
    for c in range(N_CORES):
        ds = slice(DS * c, DS * (c + 1))
        ei2s = np.stack([
            ei2[0] - I2P * c,                    # heads, shifted (match 0..5)
            (ei2[1] - I2P * c) % N,              # tails, rotated
        ]).astype(np.int32)
        in_maps.append({
            "Wns": np.ascontiguousarray(a["Wn"][ds, :]),
            "Wes": np.ascontiguousarray(a["We"][ds, :]),
            "gw": a["global_weight"].reshape(1, D),
            "bns": np.ascontiguousarray(a["bn"][ds].reshape(DS, 1)),
            "bes": np.ascontiguousarray(a["be"][ds].reshape(DS, 1)),
            "x1s": np.ascontiguousarray(a["x1"][:, ds]),
            "x2s": np.ascontiguousarray(a["x2"][:, ds]),
            "ef1s": np.ascontiguousarray(a["ef1"][:, ds]),
            "ef2s": np.ascontiguousarray(a["ef2"][:, ds]),
            "ei1": a["edge_index1"].astype(np.int32),
            "ei2s": ei2s,
            "sel": np.ascontiguousarray(eye[:, I2P * c:I2P * (c + 1)]),
        })
    return in_maps


def kernel(**inputs) -> np.ndarray:
    global LAST_RESULTS
    nc = _build()
    a = {k: np.ascontiguousarray(np.asarray(v)) for k, v in inputs.items()}
    in_maps = _make_in_maps(a)
    res = run_bass_kernel_spmd(nc, in_maps, core_ids=list(range(N_CORES)))
    LAST_RESULTS = res

    parts = []
    for c in range(N_CORES):
        o = res.results[c]["out"].reshape(ROWS, N, N)
        parts.append(np.roll(o, I2P * c, axis=1).reshape(ROWS, COLS))
    return np.concatenate(parts, axis=0).astype(np.float32)


if __name__ == "__main__":
    _build()
    print("build OK")
